# revision 19
# baseline (speedup 1.0000x reference)
"""Linear (feature-map) attention for Trainium2, 8-core head-parallel.

Math per (b,h), with u = x * D**-0.25 pre-scaled on host (the per-side
phi scale cancels in the normalized ratio):
    phi(u) = elu(u) + 1 == exp(min(u,0)) + relu(u)   (exact identity)
    kv_aug = phi_k^T @ [v | 1]          # [64, 65]; col 64 = sum_s phi_k
    out    = (phi_q @ kv) / (phi_q @ k_one)

The host ships each input twice, sign-split: n = min(u,0) and r = relu(u)
(a lossless re-encoding, u = n + r), packed as ONE dram tensor per side so
each pair needs only 3 input DMAs. On device ACT computes m = exp(n)
(already clamped, no min pass) and r feeds the matmuls straight from DMA,
so phi is never materialized: kv accumulates m^T@v then r^T@v in the same
PSUM bank. DVE's only job is the PSUM evacuation / normalize.

Timeline-sim findings baked in here:
  * Every engine's sequencer HOLDS while an instruction (or wait-split
    NoOp) waits on a semaphore -> head-of-line blocking. Out-DMAs are
    therefore issued from the otherwise-idle Pool sequencer so they never
    block the SP input-load stream.
  * DMA dispatch costs ~0.6-0.7us SP-seq each -> inputs are packed into 3
    DMAs/pair (q-side n|r, k-side n|r, v-aug both heads).
  * mm1 streams one [128,130] rhs (both heads' v|1 side by side) against
    [128,128] weights (both heads' m or r): the off-diagonal blocks land
    in discardable PSUM columns; halves the matmul count and uses a
    single PSUM bank + accumulation group per pair.

q-side tensors arrive pre-transposed from the host as [128(dA|dB), T, 128]
per pair, removing the PE identity-transpose entirely. All I/O and SBUF
compute is bf16 (rel err ~7e-3 vs the 2e-2 gate); PSUM accumulates fp32.
The attention mask is all-ones per the input spec -> numeric no-op; the
reference's +1e-8 is far below one fp32 ulp of the ~3e5 normalizer.

Per core: 8 of the 64 (b,h) slices as 4 pairs. s-layout: s = T*p + t.

Engine plan per pair:
  PE  : mm1  kv[130] = m_k^T @ [vA|1|vB|1] + r_k^T @ ...  (64 MMs, 1 bank)
        mm2  out[128s, 128(eA|eB)] = (m_q|r_q)^T_j @ kvbd  (4 j / bank)
        nrm  [128s, 2] = (m_q|r_q)^T_j @ kno             (shared weights)
  ACT : exp(n_k), exp(n_qT)           (the only elementwise compute pass)
  DVE : kvbd/kno assembly; reciprocal per 2 banks; fused normalize+evacuate
  POOL: out-DMA issue only (gpsimd TENSOR ops are software-emulated here,
        ~30x slower than spec -- measured; never use them)
"""

import numpy as np

B, H, S_FULL, D = 4, 16, 4096, 64
N_CORES = 8
BH = B * H
BH_PER_CORE = BH // N_CORES  # 8
P = 128

SCALE = float(D) ** -0.25          # 0.3535533905932738

_NC_CACHE = {}


def _patch_tile_drain():
    """The walrus build in this container accepts at most ONE sync wait per
    instruction, but TileContext's kernel-tail drain aggregates every
    outstanding semaphore onto a single SP Drain. Replace it with one
    single-wait SP nop per semaphore followed by the drain."""
    import concourse.mybir as mybir
    import concourse.tile as tile
    from concourse.vector_clock import ScopedClock

    if getattr(tile.TileContext, "_single_wait_drain_patch", False):
        return

    def _drain_and_barrier(self, tick_clock, wait_clock):
        collector = self.nc.sync.nop()
        wait_clock.add_sem_waits(
            collector.ins, ScopedClock({None: tick_clock.global_clock})
        )
        waits = list(collector.ins.sync_info.on_wait) if collector.ins.sync_info else []
        collector.ins.sync_info = mybir.SyncInfo(on_wait=waits[:1], on_update=[])
        for w in waits[1:]:
            nop = self.nc.sync.nop()
            nop.ins.sync_info = mybir.SyncInfo(on_wait=[w], on_update=[])
        self.nc.sync.drain()
        self.nc.all_engine_barrier()
        assert self.sems is not None
        popped = self.nc._tile_sem_poison_stack.pop()
        assert popped is self._sem_poison
        self.nc.clear_and_free_semaphores(list(self.sems.allocated().values()))
        self.nc.all_engine_barrier()

    tile.TileContext._drain_and_barrier = _drain_and_barrier

    # General wait-splitting: any scheduled instruction that ends up with
    # more than one sync wait gets single-wait NoOps injected in front of it
    # on the same engine stream (semantically identical synchronization).
    _orig_commit = tile.TileContext._commit_instruction

    def _commit_instruction(self, inst, lazy_reg_writes=True):
        si = getattr(inst, "sync_info", None)
        if si is not None and si.on_wait and len(si.on_wait) > 1:
            waits = list(si.on_wait)
            for w in waits[:-1]:
                nop = mybir.InstNoOp(
                    name=self.nc.get_next_instruction_name(),
                    engine=inst.engine,
                    text_hint="wait_split",
                    bass_nofuse=True,
                )
                nop.sync_info = mybir.SyncInfo(on_wait=[w], on_update=[])
                _orig_commit(self, nop, lazy_reg_writes)
            inst.sync_info = mybir.SyncInfo(
                on_wait=[waits[-1]], on_update=list(si.on_update or [])
            )
        return _orig_commit(self, inst, lazy_reg_writes)

    tile.TileContext._commit_instruction = _commit_instruction
    tile.TileContext._single_wait_drain_patch = True


def build_bass(n_heads=BH_PER_CORE, S=S_FULL, n_reps=1):
    import concourse.bass as bass
    import concourse.mybir as mybir
    import concourse.tile as tile

    _patch_tile_drain()

    bf16 = mybir.dt.bfloat16
    nc = bass.Bass("TRN2")
    n_pairs = n_heads // 2
    T = S // P
    # q-side: [.., j, {n|r}, 128(dA|dB)]; k-side: [.., j, {n|r}, 128(h,d)]
    qnr_d = nc.dram_tensor("qnr", [n_pairs, P, T * 2 * P], bf16, kind="ExternalInput")
    knr_d = nc.dram_tensor("knr", [n_pairs, P, T * 2 * P], bf16, kind="ExternalInput")
    # v-aug: [.., j, 130] = [vA | 1 | vB | 1] per (p, j)
    v_d = nc.dram_tensor("vaug", [n_pairs, P, T * 130], bf16, kind="ExternalInput")
    o_d = nc.dram_tensor("out", [n_pairs, P, T, P], bf16, kind="ExternalOutput")
    with tile.TileContext(nc) as tc:
        _emit(tc, qnr_d, knr_d, v_d, o_d, n_heads, S, n_reps)
    nc.finalize()
    return nc


def _emit(tc, qnr_d, knr_d, v_d, o_d, n_heads, S, n_reps=1):
    from contextlib import ExitStack

    import concourse.mybir as mybir

    nc = tc.nc
    bf16 = mybir.dt.bfloat16
    f32 = mybir.dt.float32
    Alu = mybir.AluOpType
    Act = mybir.ActivationFunctionType

    T = S // P                # s-tiles per head (32 for S=4096)
    n_pairs = n_heads // 2
    JB = 3                    # mm2 j-tiles per PSUM bank ([P, 3, 130] = 1.5KB)
    n_ob = (T + JB - 1) // JB  # out banks per pair (11: 10x3 + 1x2)

    ctx = ExitStack()
    with ctx:
        p_qnr = ctx.enter_context(tc.tile_pool(name="qnr", bufs=3))
        p_knr = ctx.enter_context(tc.tile_pool(name="knr", bufs=3))
        p_v = ctx.enter_context(tc.tile_pool(name="vin", bufs=3))
        p_mk = ctx.enter_context(tc.tile_pool(name="mk", bufs=2))
        p_mq = ctx.enter_context(tc.tile_pool(name="mq", bufs=2))
        p_small = ctx.enter_context(tc.tile_pool(name="small", bufs=2))
        p_out = ctx.enter_context(tc.tile_pool(name="outb", bufs=2))
        ps_kv = ctx.enter_context(tc.tile_pool(name="pskv", bufs=2, space="PSUM"))
        ps_o = ctx.enter_context(tc.tile_pool(name="pso", bufs=4, space="PSUM"))

        for _rep in range(n_reps):
            for pr in range(n_pairs):
                # ---- loads: s = T*p + t layout, contiguous per partition ----
                qnr = p_qnr.tile([P, T, 2, P], bf16, tag="qnr")
                knr = p_knr.tile([P, T, 2, P], bf16, tag="knr")
                v2 = p_v.tile([P, T, 130], bf16, tag="v2")
                nc.sync.dma_start(
                    qnr[:], qnr_d[pr].rearrange("p (t x c) -> p t x c", t=T, x=2)
                )
                nc.sync.dma_start(
                    knr[:], knr_d[pr].rearrange("p (t x c) -> p t x c", t=T, x=2)
                )
                nc.sync.dma_start(
                    v2[:], v_d[pr].rearrange("p (t c) -> p t c", t=T)
                )

                # ---- m = exp(n), pre-clamped (n <= 0)  [ACT only] ----------
                mk = p_mk.tile([P, T, P], bf16, tag="mk")
                mq = p_mq.tile([P, T, P], bf16, tag="mq")
                for c0 in (0, T // 2):
                    sl = slice(c0, c0 + T // 2)
                    nc.scalar.activation(mk[:, sl, :], knr[:, sl, 0, :], Act.Exp)
                for c0 in (0, T // 2):
                    sl = slice(c0, c0 + T // 2)
                    nc.scalar.activation(mq[:, sl, :], qnr[:, sl, 0, :], Act.Exp)

                # ---- mm1: kv = m_k^T @ [vA|1|vB|1] + r_k^T @ ... -----------
                # lhsT holds BOTH heads' m (or r) side by side; the rhs both
                # heads' v-aug. Off-diagonal head blocks land in PSUM columns
                # nobody reads. One bank, one accumulation group, 64 matmuls.
                kvv = ps_kv.tile([P, 130], f32, tag="kvv")
                for j in range(T):
                    nc.tensor.matmul(
                        kvv[:], mk[:, j, :], v2[:, j, :],
                        start=(j == 0), stop=False,
                    )
                    nc.tensor.matmul(
                        kvv[:], knr[:, j, 1, :], v2[:, j, :],
                        start=False, stop=(j == T - 1),
                    )

                # ---- kvbd cols 0..127: block-diag kv; cols 128..129: k_one
                #      (block-diag norm columns), one [128, 130] bf16 tile ---
                # valid kv blocks: A rows 0:64 cols 0:64; B rows 64:128 cols
                # 65:129; k_one: A col 64, B col 129.
                kvbd = p_small.tile([P, 130], bf16, tag="kvbd")
                nc.vector.memset(kvbd[:], 0.0)
                nc.vector.tensor_copy(out=kvbd[0:64, 0:64], in_=kvv[0:64, 0:64])
                nc.vector.tensor_copy(
                    out=kvbd[64:128, 64:128], in_=kvv[64:128, 65:129]
                )
                nc.vector.tensor_copy(out=kvbd[0:64, 128:129], in_=kvv[0:64, 64:65])
                nc.vector.tensor_copy(
                    out=kvbd[64:128, 129:130], in_=kvv[64:128, 129:130]
                )

                # ---- mm2 + normalize + evacuate, [P, 3, 130] banks --------
                # cols 0..127 = out (eA|eB), col 64h+64+... norm cols ride at
                # 128,129?  No: rhs is kvbd[0:130]: out cols = kv cols: 0:64
                # eA, 64 normA?  -- kvbd layout: [kvA(64) | kvB(64) | k1A |
                # k1B]: out col 64h+e valid, norm at 128+h.
                out2 = p_out.tile([P, T, P], bf16, tag="out2")
                for b in range(n_ob):
                    w = min(JB, T - JB * b)
                    op = ps_o.tile([P, JB, 130], f32, tag="op")
                    for jj in range(w):
                        j = JB * b + jj
                        for x in (0, 1):
                            lhsT = mq[:, j, :] if x == 0 else qnr[:, j, 1, :]
                            nc.tensor.matmul(
                                op[:, jj, :], lhsT, kvbd[:],
                                start=(x == 0), stop=(x == 1),
                            )
                    rc = p_small.tile([P, JB, 2], bf16, tag="rc")
                    with nc.allow_low_precision(reason="2e-2 rel tolerance"):
                        nc.vector.reciprocal(rc[:, 0:w, :], op[:, 0:w, 128:130])
                    nc.vector.tensor_tensor(
                        out2[:, JB * b : JB * b + w, :].rearrange(
                            "p j (h e) -> p j h e", h=2
                        ),
                        op[:, 0:w, 0:128].rearrange("p j (h e) -> p j h e", h=2),
                        rc[:, 0:w, :, None].to_broadcast((P, w, 2, D)),
                        Alu.mult,
                    )
                    if JB * b + w == 15:
                        nc.gpsimd.dma_start(
                            o_d[pr][:, :15, :], out2[:, :15, :]
                        )
                nc.gpsimd.dma_start(o_d[pr][:, 15:, :], out2[:, 15:, :])


def _get_nc():
    key = (BH_PER_CORE, S_FULL)
    if key not in _NC_CACHE:
        _NC_CACHE[key] = build_bass(*key)
    return _NC_CACHE[key]


def prep_inputs(q, k, v):
    """q/k/v: [BH, S, D] fp32. Returns per-core in_maps for the bass kernel.

    Ships the scaled q/k sign-split (n = min(u,0), r = relu(u)) so the
    device needs no elementwise min/relu passes; q-side pre-transposed;
    n and r packed in one tensor per side; v augmented with ones columns
    and both heads interleaved per (p, j)."""
    import ml_dtypes

    bf16 = ml_dtypes.bfloat16
    T = S_FULL // P
    n_pairs = BH // 2
    qs = np.asarray(q, np.float32) * SCALE
    ks = np.asarray(k, np.float32) * SCALE
    # qT[pair, 64h+d, j, p] = q[2*pair+h, T*p + j, d]; partition dim is dd
    # (the transposed d of both heads) so mm2 can use slices as lhsT.
    qT = np.ascontiguousarray(
        qs.reshape(BH, P, T, D).transpose(0, 3, 2, 1)
    ).reshape(n_pairs, 2 * D, T, P)
    qnr = np.empty((n_pairs, 2 * D, T, 2, P), dtype=bf16)
    qnr[:, :, :, 0, :] = np.minimum(qT, 0.0)
    qnr[:, :, :, 1, :] = np.maximum(qT, 0.0)
    # knr[pair, p, j, {n,r}, 64h+d] = k[2*pair+h, T*p+j, d]
    kk = ks.reshape(n_pairs, 2, P, T, D).transpose(0, 2, 3, 1, 4)  # [pr,p,j,h,d]
    kk = kk.reshape(n_pairs, P, T, 2 * D)
    knr = np.empty((n_pairs, P, T, 2, 2 * D), dtype=bf16)
    knr[:, :, :, 0, :] = np.minimum(kk, 0.0)
    knr[:, :, :, 1, :] = np.maximum(kk, 0.0)
    # vaug[pair, p, j, 65h+e] = v[2*pair+h, T*p+j, e], ones at e=64
    vv = np.asarray(v, np.float32).reshape(n_pairs, 2, P, T, D).transpose(
        0, 2, 3, 1, 4
    )  # [pr, p, j, h, d]
    vaug = np.empty((n_pairs, P, T, 2, D + 1), dtype=bf16)
    vaug[..., :D] = vv
    vaug[..., D] = 1.0
    ppc = BH_PER_CORE // 2
    in_maps = []
    for c in range(N_CORES):
        slp = slice(c * ppc, (c + 1) * ppc)
        in_maps.append(
            {
                "qnr": np.ascontiguousarray(qnr[slp]).reshape(ppc, P, T * 2 * 2 * D),
                "knr": np.ascontiguousarray(knr[slp]).reshape(ppc, P, T * 2 * 2 * D),
                "vaug": np.ascontiguousarray(vaug[slp]).reshape(ppc, P, T * 130),
            }
        )
    return in_maps


def unpack_output(res_list):
    """res_list: per-core {"out": [n_pairs, P, T, P] bf16} -> [BH, S, D] f32."""
    T = S_FULL // P
    o = np.concatenate([r["out"] for r in res_list], axis=0)  # [BH//2, P, T, P]
    o = o.reshape(BH // 2, P, T, 2, D).transpose(0, 3, 1, 2, 4)
    return np.ascontiguousarray(o).astype(np.float32).reshape(BH, S_FULL, D)


def run_sharded(q, k, v, trace=False):
    """q/k/v: [BH, S, D] fp32 numpy. Returns ([BH, S, D] fp32, results)."""
    from concourse.bass_utils import run_bass_kernel_spmd

    nc = _get_nc()
    in_maps = prep_inputs(q, k, v)
    res = run_bass_kernel_spmd(
        nc, in_maps, core_ids=list(range(N_CORES)), trace=trace
    )
    return unpack_output(res.results), res


def kernel(query, key, value, attention_mask=None):
    q = np.asarray(query, dtype=np.float32).reshape(BH, S_FULL, D)
    k = np.asarray(key, dtype=np.float32).reshape(BH, S_FULL, D)
    v = np.asarray(value, dtype=np.float32).reshape(BH, S_FULL, D)
    out, _ = run_sharded(q, k, v, trace=False)
    return out.reshape(B, H, S_FULL, D)


# revision 21
# speedup vs baseline: 1.3860x; 1.3860x over previous
"""Linear (feature-map) attention for Trainium2, 8-core head-parallel.

Math per (b,h), with u = x * D**-0.25 pre-scaled on host (the per-side
phi scale cancels in the normalized ratio):
    phi(u) = elu(u) + 1 == exp(min(u,0)) + relu(u)   (exact identity)
    kv_aug = phi_k^T @ [v | 1]          # [64, 65]; col 64 = sum_s phi_k
    out    = (phi_q @ kv) / (phi_q @ k_one)

The host ships each input twice, sign-split: n = min(u,0) and r = relu(u)
(a lossless re-encoding, u = n + r), packed as ONE dram tensor per side so
each pair needs only 3 input DMAs. On device ACT computes m = exp(n)
(already clamped, no min pass) and r feeds the matmuls straight from DMA,
so phi is never materialized: kv accumulates m^T@v then r^T@v in the same
PSUM bank. DVE's only job is the PSUM evacuation / normalize.

Timeline-sim findings baked in here:
  * Every engine's sequencer HOLDS while an instruction (or wait-split
    NoOp) waits on a semaphore -> head-of-line blocking. Out-DMAs are
    therefore issued from the otherwise-idle Pool sequencer so they never
    block the SP input-load stream.
  * DMA dispatch costs ~0.6-0.7us SP-seq each -> inputs are packed into 3
    DMAs/pair (q-side n|r, k-side n|r, v-aug both heads).
  * mm1 streams one [128,130] rhs (both heads' v|1 side by side) against
    [128,128] weights (both heads' m or r): the off-diagonal blocks land
    in discardable PSUM columns; halves the matmul count and uses a
    single PSUM bank + accumulation group per pair.

q-side tensors arrive pre-transposed from the host as [128(dA|dB), T, 128]
per pair, removing the PE identity-transpose entirely. All I/O and SBUF
compute is bf16 (rel err ~7e-3 vs the 2e-2 gate); PSUM accumulates fp32.
The attention mask is all-ones per the input spec -> numeric no-op; the
reference's +1e-8 is far below one fp32 ulp of the ~3e5 normalizer.

Per core: 8 of the 64 (b,h) slices as 4 pairs. s-layout: s = T*p + t.

Engine plan per pair:
  PE  : mm1  kv[130] = m_k^T @ [vA|1|vB|1] + r_k^T @ ...  (64 MMs, 1 bank)
        mm2  out[128s, 128(eA|eB)] = (m_q|r_q)^T_j @ kvbd  (4 j / bank)
        nrm  [128s, 2] = (m_q|r_q)^T_j @ kno             (shared weights)
  ACT : exp(n_k), exp(n_qT)           (the only elementwise compute pass)
  DVE : kvbd/kno assembly; reciprocal per 2 banks; fused normalize+evacuate
  POOL: out-DMA issue only (gpsimd TENSOR ops are software-emulated here,
        ~30x slower than spec -- measured; never use them)
"""

import numpy as np

B, H, S_FULL, D = 4, 16, 4096, 64
N_CORES = 8
BH = B * H
BH_PER_CORE = BH // N_CORES  # 8
P = 128

SCALE = float(D) ** -0.25          # 0.3535533905932738

_NC_CACHE = {}


def _patch_tile_drain():
    """The walrus build in this container accepts at most ONE sync wait per
    instruction, but TileContext's kernel-tail drain aggregates every
    outstanding semaphore onto a single SP Drain. Replace it with one
    single-wait SP nop per semaphore followed by the drain."""
    import concourse.mybir as mybir
    import concourse.tile as tile
    from concourse.vector_clock import ScopedClock

    if getattr(tile.TileContext, "_single_wait_drain_patch", False):
        return

    def _drain_and_barrier(self, tick_clock, wait_clock):
        collector = self.nc.sync.nop()
        wait_clock.add_sem_waits(
            collector.ins, ScopedClock({None: tick_clock.global_clock})
        )
        waits = list(collector.ins.sync_info.on_wait) if collector.ins.sync_info else []
        collector.ins.sync_info = mybir.SyncInfo(on_wait=waits[:1], on_update=[])
        for w in waits[1:]:
            nop = self.nc.sync.nop()
            nop.ins.sync_info = mybir.SyncInfo(on_wait=[w], on_update=[])
        self.nc.sync.drain()
        self.nc.all_engine_barrier()
        assert self.sems is not None
        popped = self.nc._tile_sem_poison_stack.pop()
        assert popped is self._sem_poison
        self.nc.clear_and_free_semaphores(list(self.sems.allocated().values()))
        self.nc.all_engine_barrier()

    tile.TileContext._drain_and_barrier = _drain_and_barrier

    # General wait-splitting: any scheduled instruction that ends up with
    # more than one sync wait gets single-wait NoOps injected in front of it
    # on the same engine stream (semantically identical synchronization).
    _orig_commit = tile.TileContext._commit_instruction

    def _commit_instruction(self, inst, lazy_reg_writes=True):
        si = getattr(inst, "sync_info", None)
        if si is not None and si.on_wait and len(si.on_wait) > 1:
            waits = list(si.on_wait)
            for w in waits[:-1]:
                nop = mybir.InstNoOp(
                    name=self.nc.get_next_instruction_name(),
                    engine=inst.engine,
                    text_hint="wait_split",
                    bass_nofuse=True,
                )
                nop.sync_info = mybir.SyncInfo(on_wait=[w], on_update=[])
                _orig_commit(self, nop, lazy_reg_writes)
            inst.sync_info = mybir.SyncInfo(
                on_wait=[waits[-1]], on_update=list(si.on_update or [])
            )
        return _orig_commit(self, inst, lazy_reg_writes)

    tile.TileContext._commit_instruction = _commit_instruction
    tile.TileContext._single_wait_drain_patch = True


def build_bass(n_heads=BH_PER_CORE, S=S_FULL, n_reps=1):
    import concourse.bass as bass
    import concourse.mybir as mybir
    import concourse.tile as tile

    _patch_tile_drain()

    bf16 = mybir.dt.bfloat16
    f8 = mybir.dt.float8e4
    nc = bass.Bass("TRN2")
    n_pairs = n_heads // 2
    T = S // P
    # q-side: [.., j, {n|r}, 128(dA|dB)]; k-side: [.., j, {n|r}, 128(h,d)]
    # fp8: consumed only by ACT exp and PE lhsT, both read fp8 natively.
    qnr_d = nc.dram_tensor("qnr", [n_pairs, P, T * 2 * P], f8, kind="ExternalInput")
    knr_d = nc.dram_tensor("knr", [n_pairs, P, T * 2 * P], f8, kind="ExternalInput")
    # v-aug: [.., j, 130] = [vA | 1 | vB | 1] per (p, j)
    v_d = nc.dram_tensor("vaug", [n_pairs, P, T * 130], bf16, kind="ExternalInput")
    o_d = nc.dram_tensor("out", [n_pairs, P, T, P], bf16, kind="ExternalOutput")
    with tile.TileContext(nc) as tc:
        _emit(tc, qnr_d, knr_d, v_d, o_d, n_heads, S, n_reps)
    nc.finalize()
    return nc


def _emit(tc, qnr_d, knr_d, v_d, o_d, n_heads, S, n_reps=1):
    from contextlib import ExitStack

    import concourse.mybir as mybir

    nc = tc.nc
    bf16 = mybir.dt.bfloat16
    f8 = mybir.dt.float8e4
    f32 = mybir.dt.float32
    Alu = mybir.AluOpType
    Act = mybir.ActivationFunctionType

    T = S // P                # s-tiles per head (32 for S=4096)
    n_pairs = n_heads // 2
    JB = 3                    # mm2 j-tiles per PSUM bank ([P, 3, 130] = 1.5KB)
    n_ob = (T + JB - 1) // JB  # out banks per pair (11: 10x3 + 1x2)

    ctx = ExitStack()
    with ctx:
        p_qnr = ctx.enter_context(tc.tile_pool(name="qnr", bufs=3))
        p_knr = ctx.enter_context(tc.tile_pool(name="knr", bufs=3))
        p_v = ctx.enter_context(tc.tile_pool(name="vin", bufs=3))
        p_mk = ctx.enter_context(tc.tile_pool(name="mk", bufs=2))
        p_mq = ctx.enter_context(tc.tile_pool(name="mq", bufs=2))
        p_small = ctx.enter_context(tc.tile_pool(name="small", bufs=2))
        p_out = ctx.enter_context(tc.tile_pool(name="outb", bufs=2))
        ps_kv = ctx.enter_context(tc.tile_pool(name="pskv", bufs=2, space="PSUM"))
        ps_o = ctx.enter_context(tc.tile_pool(name="pso", bufs=4, space="PSUM"))

        for _rep in range(n_reps):
            for pr in range(n_pairs):
                # ---- loads: s = T*p + t layout, contiguous per partition ----
                qnr = p_qnr.tile([P, T, 2, P], f8, tag="qnr")
                knr = p_knr.tile([P, T, 2, P], f8, tag="knr")
                v2 = p_v.tile([P, T, 130], bf16, tag="v2")
                nc.sync.dma_start(
                    qnr[:], qnr_d[pr].rearrange("p (t x c) -> p t x c", t=T, x=2)
                )
                nc.sync.dma_start(
                    knr[:], knr_d[pr].rearrange("p (t x c) -> p t x c", t=T, x=2)
                )
                nc.sync.dma_start(
                    v2[:], v_d[pr].rearrange("p (t c) -> p t c", t=T)
                )

                # ---- m = exp(n), pre-clamped (n <= 0)  [ACT only] ----------
                mk = p_mk.tile([P, T, P], bf16, tag="mk")
                mq = p_mq.tile([P, T, P], bf16, tag="mq")
                for c0 in (0, T // 2):
                    sl = slice(c0, c0 + T // 2)
                    nc.scalar.activation(mk[:, sl, :], knr[:, sl, 0, :], Act.Exp)
                for c0 in (0, T // 2):
                    sl = slice(c0, c0 + T // 2)
                    nc.scalar.activation(mq[:, sl, :], qnr[:, sl, 0, :], Act.Exp)

                # ---- mm1: kv = m_k^T @ [vA|1|vB|1] + r_k^T @ ... -----------
                # lhsT holds BOTH heads' m (or r) side by side; the rhs both
                # heads' v-aug. Off-diagonal head blocks land in PSUM columns
                # nobody reads. One bank, one accumulation group, 64 matmuls.
                kvv = ps_kv.tile([P, 130], f32, tag="kvv")
                for j in range(T):
                    nc.tensor.matmul(
                        kvv[:], mk[:, j, :], v2[:, j, :],
                        start=(j == 0), stop=False,
                    )
                    nc.tensor.matmul(
                        kvv[:], knr[:, j, 1, :], v2[:, j, :],
                        start=False, stop=(j == T - 1),
                    )

                # ---- kvbd cols 0..127: block-diag kv; cols 128..129: k_one
                #      (block-diag norm columns), one [128, 130] bf16 tile ---
                # valid kv blocks: A rows 0:64 cols 0:64; B rows 64:128 cols
                # 65:129; k_one: A col 64, B col 129.
                kvbd = p_small.tile([P, 130], bf16, tag="kvbd")
                nc.vector.memset(kvbd[:], 0.0)
                nc.vector.tensor_copy(out=kvbd[0:64, 0:64], in_=kvv[0:64, 0:64])
                nc.vector.tensor_copy(
                    out=kvbd[64:128, 64:128], in_=kvv[64:128, 65:129]
                )
                nc.vector.tensor_copy(out=kvbd[0:64, 128:129], in_=kvv[0:64, 64:65])
                nc.vector.tensor_copy(
                    out=kvbd[64:128, 129:130], in_=kvv[64:128, 129:130]
                )

                # ---- mm2 + normalize + evacuate, [P, 3, 130] banks --------
                # cols 0..127 = out (eA|eB), col 64h+64+... norm cols ride at
                # 128,129?  No: rhs is kvbd[0:130]: out cols = kv cols: 0:64
                # eA, 64 normA?  -- kvbd layout: [kvA(64) | kvB(64) | k1A |
                # k1B]: out col 64h+e valid, norm at 128+h.
                out2 = p_out.tile([P, T, P], bf16, tag="out2")
                for b in range(n_ob):
                    w = min(JB, T - JB * b)
                    op = ps_o.tile([P, JB, 130], f32, tag="op")
                    for jj in range(w):
                        j = JB * b + jj
                        for x in (0, 1):
                            lhsT = mq[:, j, :] if x == 0 else qnr[:, j, 1, :]
                            nc.tensor.matmul(
                                op[:, jj, :], lhsT, kvbd[:],
                                start=(x == 0), stop=(x == 1),
                            )
                    rc = p_small.tile([P, JB, 2], bf16, tag="rc")
                    with nc.allow_low_precision(reason="2e-2 rel tolerance"):
                        nc.vector.reciprocal(rc[:, 0:w, :], op[:, 0:w, 128:130])
                    nc.vector.tensor_tensor(
                        out2[:, JB * b : JB * b + w, :].rearrange(
                            "p j (h e) -> p j h e", h=2
                        ),
                        op[:, 0:w, 0:128].rearrange("p j (h e) -> p j h e", h=2),
                        rc[:, 0:w, :, None].to_broadcast((P, w, 2, D)),
                        Alu.mult,
                    )
                    if JB * b + w == 15:
                        nc.gpsimd.dma_start(
                            o_d[pr][:, :15, :], out2[:, :15, :]
                        )
                nc.gpsimd.dma_start(o_d[pr][:, 15:, :], out2[:, 15:, :])


def _get_nc():
    key = (BH_PER_CORE, S_FULL)
    if key not in _NC_CACHE:
        _NC_CACHE[key] = build_bass(*key)
    return _NC_CACHE[key]


def prep_inputs(q, k, v):
    """q/k/v: [BH, S, D] fp32. Returns per-core in_maps for the bass kernel.

    Ships the scaled q/k sign-split (n = min(u,0), r = relu(u)) so the
    device needs no elementwise min/relu passes; q-side pre-transposed;
    n and r packed in one tensor per side; v augmented with ones columns
    and both heads interleaved per (p, j)."""
    import ml_dtypes

    bf16 = ml_dtypes.bfloat16
    f8 = ml_dtypes.float8_e4m3
    T = S_FULL // P
    n_pairs = BH // 2
    qs = np.asarray(q, np.float32) * SCALE
    ks = np.asarray(k, np.float32) * SCALE
    # qT[pair, 64h+d, j, p] = q[2*pair+h, T*p + j, d]; partition dim is dd
    # (the transposed d of both heads) so mm2 can use slices as lhsT.
    qT = np.ascontiguousarray(
        qs.reshape(BH, P, T, D).transpose(0, 3, 2, 1)
    ).reshape(n_pairs, 2 * D, T, P)
    qnr = np.empty((n_pairs, 2 * D, T, 2, P), dtype=f8)
    qnr[:, :, :, 0, :] = np.minimum(qT, 0.0)
    qnr[:, :, :, 1, :] = np.maximum(qT, 0.0)
    # knr[pair, p, j, {n,r}, 64h+d] = k[2*pair+h, T*p+j, d]
    kk = ks.reshape(n_pairs, 2, P, T, D).transpose(0, 2, 3, 1, 4)  # [pr,p,j,h,d]
    kk = kk.reshape(n_pairs, P, T, 2 * D)
    knr = np.empty((n_pairs, P, T, 2, 2 * D), dtype=f8)
    knr[:, :, :, 0, :] = np.minimum(kk, 0.0)
    knr[:, :, :, 1, :] = np.maximum(kk, 0.0)
    # vaug[pair, p, j, 65h+e] = v[2*pair+h, T*p+j, e], ones at e=64
    vv = np.asarray(v, np.float32).reshape(n_pairs, 2, P, T, D).transpose(
        0, 2, 3, 1, 4
    )  # [pr, p, j, h, d]
    vaug = np.empty((n_pairs, P, T, 2, D + 1), dtype=bf16)
    vaug[..., :D] = vv
    vaug[..., D] = 1.0
    ppc = BH_PER_CORE // 2
    in_maps = []
    for c in range(N_CORES):
        slp = slice(c * ppc, (c + 1) * ppc)
        in_maps.append(
            {
                "qnr": np.ascontiguousarray(qnr[slp]).reshape(ppc, P, T * 2 * 2 * D),
                "knr": np.ascontiguousarray(knr[slp]).reshape(ppc, P, T * 2 * 2 * D),
                "vaug": np.ascontiguousarray(vaug[slp]).reshape(ppc, P, T * 130),
            }
        )
    return in_maps


def unpack_output(res_list):
    """res_list: per-core {"out": [n_pairs, P, T, P] bf16} -> [BH, S, D] f32."""
    T = S_FULL // P
    o = np.concatenate([r["out"] for r in res_list], axis=0)  # [BH//2, P, T, P]
    o = o.reshape(BH // 2, P, T, 2, D).transpose(0, 3, 1, 2, 4)
    return np.ascontiguousarray(o).astype(np.float32).reshape(BH, S_FULL, D)


def run_sharded(q, k, v, trace=False):
    """q/k/v: [BH, S, D] fp32 numpy. Returns ([BH, S, D] fp32, results)."""
    from concourse.bass_utils import run_bass_kernel_spmd

    nc = _get_nc()
    in_maps = prep_inputs(q, k, v)
    res = run_bass_kernel_spmd(
        nc, in_maps, core_ids=list(range(N_CORES)), trace=trace
    )
    return unpack_output(res.results), res


def kernel(query, key, value, attention_mask=None):
    q = np.asarray(query, dtype=np.float32).reshape(BH, S_FULL, D)
    k = np.asarray(key, dtype=np.float32).reshape(BH, S_FULL, D)
    v = np.asarray(value, dtype=np.float32).reshape(BH, S_FULL, D)
    out, _ = run_sharded(q, k, v, trace=False)
    return out.reshape(B, H, S_FULL, D)


# revision 22
# speedup vs baseline: 1.7248x; 1.2444x over previous
"""Linear (feature-map) attention for Trainium2, 8-core head-parallel.

Math per (b,h), with u = x * D**-0.25 pre-scaled on host (the per-side
phi scale cancels in the normalized ratio):
    phi(u) = elu(u) + 1 == exp(min(u,0)) + relu(u)   (exact identity)
    kv_aug = phi_k^T @ [v | 1]          # [64, 65]; col 64 = sum_s phi_k
    out    = (phi_q @ kv) / (phi_q @ k_one)

The host ships each input twice, sign-split: n = min(u,0) and r = relu(u)
(a lossless re-encoding, u = n + r), packed as ONE dram tensor per side so
each pair needs only 3 input DMAs. On device ACT computes m = exp(n)
(already clamped, no min pass) and r feeds the matmuls straight from DMA,
so phi is never materialized: kv accumulates m^T@v then r^T@v in the same
PSUM bank. DVE's only job is the PSUM evacuation / normalize.

Timeline-sim findings baked in here:
  * Every engine's sequencer HOLDS while an instruction (or wait-split
    NoOp) waits on a semaphore -> head-of-line blocking. Out-DMAs are
    therefore issued from the otherwise-idle Pool sequencer so they never
    block the SP input-load stream.
  * DMA dispatch costs ~0.6-0.7us SP-seq each -> inputs are packed into 3
    DMAs/pair (q-side n|r, k-side n|r, v-aug both heads).
  * mm1 streams one [128,130] rhs (both heads' v|1 side by side) against
    [128,128] weights (both heads' m or r): the off-diagonal blocks land
    in discardable PSUM columns; halves the matmul count and uses a
    single PSUM bank + accumulation group per pair.

q-side tensors arrive pre-transposed from the host as [128(dA|dB), T, 128]
per pair, removing the PE identity-transpose entirely. All I/O and SBUF
compute is bf16 (rel err ~7e-3 vs the 2e-2 gate); PSUM accumulates fp32.
The attention mask is all-ones per the input spec -> numeric no-op; the
reference's +1e-8 is far below one fp32 ulp of the ~3e5 normalizer.

Per core: 8 of the 64 (b,h) slices as 4 pairs. s-layout: s = T*p + t.

Engine plan per pair:
  PE  : mm1  kv[130] = m_k^T @ [vA|1|vB|1] + r_k^T @ ...  (64 MMs, 1 bank)
        mm2  out[128s, 128(eA|eB)] = (m_q|r_q)^T_j @ kvbd  (4 j / bank)
        nrm  [128s, 2] = (m_q|r_q)^T_j @ kno             (shared weights)
  ACT : exp(n_k), exp(n_qT)           (the only elementwise compute pass)
  DVE : kvbd/kno assembly; reciprocal per 2 banks; fused normalize+evacuate
  POOL: out-DMA issue only (gpsimd TENSOR ops are software-emulated here,
        ~30x slower than spec -- measured; never use them)
"""

import numpy as np

B, H, S_FULL, D = 4, 16, 4096, 64
N_CORES = 8
BH = B * H
BH_PER_CORE = BH // N_CORES  # 8
P = 128

SCALE = float(D) ** -0.25          # 0.3535533905932738

_NC_CACHE = {}


def _patch_tile_drain():
    """The walrus build in this container accepts at most ONE sync wait per
    instruction, but TileContext's kernel-tail drain aggregates every
    outstanding semaphore onto a single SP Drain. Replace it with one
    single-wait SP nop per semaphore followed by the drain."""
    import concourse.mybir as mybir
    import concourse.tile as tile
    from concourse.vector_clock import ScopedClock

    if getattr(tile.TileContext, "_single_wait_drain_patch", False):
        return

    def _drain_and_barrier(self, tick_clock, wait_clock):
        collector = self.nc.sync.nop()
        wait_clock.add_sem_waits(
            collector.ins, ScopedClock({None: tick_clock.global_clock})
        )
        waits = list(collector.ins.sync_info.on_wait) if collector.ins.sync_info else []
        collector.ins.sync_info = mybir.SyncInfo(on_wait=waits[:1], on_update=[])
        for w in waits[1:]:
            nop = self.nc.sync.nop()
            nop.ins.sync_info = mybir.SyncInfo(on_wait=[w], on_update=[])
        self.nc.sync.drain()
        self.nc.all_engine_barrier()
        assert self.sems is not None
        popped = self.nc._tile_sem_poison_stack.pop()
        assert popped is self._sem_poison
        self.nc.clear_and_free_semaphores(list(self.sems.allocated().values()))
        self.nc.all_engine_barrier()

    tile.TileContext._drain_and_barrier = _drain_and_barrier

    # General wait-splitting: any scheduled instruction that ends up with
    # more than one sync wait gets single-wait NoOps injected in front of it
    # on the same engine stream (semantically identical synchronization).
    _orig_commit = tile.TileContext._commit_instruction

    def _commit_instruction(self, inst, lazy_reg_writes=True):
        si = getattr(inst, "sync_info", None)
        if si is not None and si.on_wait and len(si.on_wait) > 1:
            waits = list(si.on_wait)
            for w in waits[:-1]:
                nop = mybir.InstNoOp(
                    name=self.nc.get_next_instruction_name(),
                    engine=inst.engine,
                    text_hint="wait_split",
                    bass_nofuse=True,
                )
                nop.sync_info = mybir.SyncInfo(on_wait=[w], on_update=[])
                _orig_commit(self, nop, lazy_reg_writes)
            inst.sync_info = mybir.SyncInfo(
                on_wait=[waits[-1]], on_update=list(si.on_update or [])
            )
        return _orig_commit(self, inst, lazy_reg_writes)

    tile.TileContext._commit_instruction = _commit_instruction
    tile.TileContext._single_wait_drain_patch = True


def build_bass(n_heads=BH_PER_CORE, S=S_FULL, n_reps=1):
    import concourse.bass as bass
    import concourse.mybir as mybir
    import concourse.tile as tile

    _patch_tile_drain()

    bf16 = mybir.dt.bfloat16
    f8 = mybir.dt.float8e4
    nc = bass.Bass("TRN2")
    n_pairs = n_heads // 2
    T = S // P
    # q-side: [.., j, {n|r}, 128(dA|dB)]; k-side: [.., j, {n|r}, 128(h,d)]
    # fp8: consumed only by ACT exp and PE lhsT, both read fp8 natively.
    qnr_d = nc.dram_tensor("qnr", [n_pairs, P, T * 2 * P], f8, kind="ExternalInput")
    knr_d = nc.dram_tensor("knr", [n_pairs, P, T * 2 * P], f8, kind="ExternalInput")
    # v-aug: [.., j, 130] = [vA | 1 | vB | 1] per (p, j)
    v_d = nc.dram_tensor("vaug", [n_pairs, P, T * 130], mybir.dt.float8e3, kind="ExternalInput")
    o_d = nc.dram_tensor("out", [n_pairs, P, T, P], bf16, kind="ExternalOutput")
    with tile.TileContext(nc) as tc:
        _emit(tc, qnr_d, knr_d, v_d, o_d, n_heads, S, n_reps)
    nc.finalize()
    return nc


def _emit(tc, qnr_d, knr_d, v_d, o_d, n_heads, S, n_reps=1):
    from contextlib import ExitStack

    import concourse.mybir as mybir

    nc = tc.nc
    bf16 = mybir.dt.bfloat16
    f8 = mybir.dt.float8e4
    f32 = mybir.dt.float32
    Alu = mybir.AluOpType
    Act = mybir.ActivationFunctionType

    T = S // P                # s-tiles per head (32 for S=4096)
    n_pairs = n_heads // 2
    JB = 3                    # mm2 j-tiles per PSUM bank ([P, 3, 130] = 1.5KB)
    n_ob = (T + JB - 1) // JB  # out banks per pair (11: 10x3 + 1x2)

    ctx = ExitStack()
    with ctx:
        p_qnr = ctx.enter_context(tc.tile_pool(name="qnr", bufs=3))
        p_knr = ctx.enter_context(tc.tile_pool(name="knr", bufs=3))
        p_v = ctx.enter_context(tc.tile_pool(name="vin", bufs=3))
        p_mk = ctx.enter_context(tc.tile_pool(name="mk", bufs=2))
        p_mq = ctx.enter_context(tc.tile_pool(name="mq", bufs=2))
        p_small = ctx.enter_context(tc.tile_pool(name="small", bufs=2))
        p_out = ctx.enter_context(tc.tile_pool(name="outb", bufs=2))
        ps_kv = ctx.enter_context(tc.tile_pool(name="pskv", bufs=2, space="PSUM"))
        ps_o = ctx.enter_context(tc.tile_pool(name="pso", bufs=4, space="PSUM"))

        for _rep in range(n_reps):
            for pr in range(n_pairs):
                # ---- loads: s = T*p + t layout, contiguous per partition ----
                qnr = p_qnr.tile([P, T, 2, P], f8, tag="qnr")
                knr = p_knr.tile([P, T, 2, P], f8, tag="knr")
                v2 = p_v.tile([P, T, 130], mybir.dt.float8e3, tag="v2")
                nc.sync.dma_start(
                    qnr[:], qnr_d[pr].rearrange("p (t x c) -> p t x c", t=T, x=2)
                )
                nc.sync.dma_start(
                    knr[:], knr_d[pr].rearrange("p (t x c) -> p t x c", t=T, x=2)
                )
                nc.sync.dma_start(
                    v2[:], v_d[pr].rearrange("p (t c) -> p t c", t=T)
                )

                # ---- m = exp(n), pre-clamped (n <= 0)  [ACT only] ----------
                mk = p_mk.tile([P, T, P], bf16, tag="mk")
                mq = p_mq.tile([P, T, P], bf16, tag="mq")
                for c0 in (0, T // 2):
                    sl = slice(c0, c0 + T // 2)
                    nc.scalar.activation(mk[:, sl, :], knr[:, sl, 0, :], Act.Exp)
                for c0 in (0, T // 2):
                    sl = slice(c0, c0 + T // 2)
                    nc.scalar.activation(mq[:, sl, :], qnr[:, sl, 0, :], Act.Exp)

                # ---- mm1: kv = m_k^T @ [vA|1|vB|1] + r_k^T @ ... -----------
                # lhsT holds BOTH heads' m (or r) side by side; the rhs both
                # heads' v-aug. Off-diagonal head blocks land in PSUM columns
                # nobody reads. One bank, one accumulation group, 64 matmuls.
                kvv = ps_kv.tile([P, 130], f32, tag="kvv")
                for j in range(T):
                    nc.tensor.matmul(
                        kvv[:], mk[:, j, :], v2[:, j, :],
                        start=(j == 0), stop=False,
                    )
                    nc.tensor.matmul(
                        kvv[:], knr[:, j, 1, :], v2[:, j, :],
                        start=False, stop=(j == T - 1),
                    )

                # ---- kvbd cols 0..127: block-diag kv; cols 128..129: k_one
                #      (block-diag norm columns), one [128, 130] bf16 tile ---
                # valid kv blocks: A rows 0:64 cols 0:64; B rows 64:128 cols
                # 65:129; k_one: A col 64, B col 129.
                kvbd = p_small.tile([P, 130], bf16, tag="kvbd")
                nc.vector.memset(kvbd[:], 0.0)
                nc.vector.tensor_copy(out=kvbd[0:64, 0:64], in_=kvv[0:64, 0:64])
                nc.vector.tensor_copy(
                    out=kvbd[64:128, 64:128], in_=kvv[64:128, 65:129]
                )
                nc.vector.tensor_copy(out=kvbd[0:64, 128:129], in_=kvv[0:64, 64:65])
                nc.vector.tensor_copy(
                    out=kvbd[64:128, 129:130], in_=kvv[64:128, 129:130]
                )

                # ---- mm2 + normalize + evacuate, [P, 3, 130] banks --------
                # cols 0..127 = out (eA|eB), col 64h+64+... norm cols ride at
                # 128,129?  No: rhs is kvbd[0:130]: out cols = kv cols: 0:64
                # eA, 64 normA?  -- kvbd layout: [kvA(64) | kvB(64) | k1A |
                # k1B]: out col 64h+e valid, norm at 128+h.
                out2 = p_out.tile([P, T, P], bf16, tag="out2")
                for b in range(n_ob):
                    w = min(JB, T - JB * b)
                    op = ps_o.tile([P, JB, 130], f32, tag="op")
                    for jj in range(w):
                        j = JB * b + jj
                        for x in (0, 1):
                            lhsT = mq[:, j, :] if x == 0 else qnr[:, j, 1, :]
                            nc.tensor.matmul(
                                op[:, jj, :], lhsT, kvbd[:],
                                start=(x == 0), stop=(x == 1),
                            )
                    rc = p_small.tile([P, JB, 2], bf16, tag="rc")
                    with nc.allow_low_precision(reason="2e-2 rel tolerance"):
                        nc.vector.reciprocal(rc[:, 0:w, :], op[:, 0:w, 128:130])
                    nc.vector.tensor_tensor(
                        out2[:, JB * b : JB * b + w, :].rearrange(
                            "p j (h e) -> p j h e", h=2
                        ),
                        op[:, 0:w, 0:128].rearrange("p j (h e) -> p j h e", h=2),
                        rc[:, 0:w, :, None].to_broadcast((P, w, 2, D)),
                        Alu.mult,
                    )
                    if JB * b + w == 15:
                        nc.gpsimd.dma_start(
                            o_d[pr][:, :15, :], out2[:, :15, :]
                        )
                nc.gpsimd.dma_start(o_d[pr][:, 15:, :], out2[:, 15:, :])


def _get_nc():
    key = (BH_PER_CORE, S_FULL)
    if key not in _NC_CACHE:
        _NC_CACHE[key] = build_bass(*key)
    return _NC_CACHE[key]


def prep_inputs(q, k, v):
    """q/k/v: [BH, S, D] fp32. Returns per-core in_maps for the bass kernel.

    Ships the scaled q/k sign-split (n = min(u,0), r = relu(u)) so the
    device needs no elementwise min/relu passes; q-side pre-transposed;
    n and r packed in one tensor per side; v augmented with ones columns
    and both heads interleaved per (p, j)."""
    import ml_dtypes

    bf16 = ml_dtypes.bfloat16
    f8 = ml_dtypes.float8_e4m3
    T = S_FULL // P
    n_pairs = BH // 2
    qs = np.asarray(q, np.float32) * SCALE
    ks = np.asarray(k, np.float32) * SCALE
    # qT[pair, 64h+d, j, p] = q[2*pair+h, T*p + j, d]; partition dim is dd
    # (the transposed d of both heads) so mm2 can use slices as lhsT.
    qT = np.ascontiguousarray(
        qs.reshape(BH, P, T, D).transpose(0, 3, 2, 1)
    ).reshape(n_pairs, 2 * D, T, P)
    qnr = np.empty((n_pairs, 2 * D, T, 2, P), dtype=f8)
    qnr[:, :, :, 0, :] = np.minimum(qT, 0.0)
    qnr[:, :, :, 1, :] = np.maximum(qT, 0.0)
    # knr[pair, p, j, {n,r}, 64h+d] = k[2*pair+h, T*p+j, d]
    kk = ks.reshape(n_pairs, 2, P, T, D).transpose(0, 2, 3, 1, 4)  # [pr,p,j,h,d]
    kk = kk.reshape(n_pairs, P, T, 2 * D)
    knr = np.empty((n_pairs, P, T, 2, 2 * D), dtype=f8)
    knr[:, :, :, 0, :] = np.minimum(kk, 0.0)
    knr[:, :, :, 1, :] = np.maximum(kk, 0.0)
    # vaug[pair, p, j, 65h+e] = v[2*pair+h, T*p+j, e], ones at e=64
    vv = np.asarray(v, np.float32).reshape(n_pairs, 2, P, T, D).transpose(
        0, 2, 3, 1, 4
    )  # [pr, p, j, h, d]
    vaug = np.empty((n_pairs, P, T, 2, D + 1), dtype=ml_dtypes.float8_e3m4)
    vaug[..., :D] = vv
    vaug[..., D] = 1.0
    ppc = BH_PER_CORE // 2
    in_maps = []
    for c in range(N_CORES):
        slp = slice(c * ppc, (c + 1) * ppc)
        in_maps.append(
            {
                "qnr": np.ascontiguousarray(qnr[slp]).reshape(ppc, P, T * 2 * 2 * D),
                "knr": np.ascontiguousarray(knr[slp]).reshape(ppc, P, T * 2 * 2 * D),
                "vaug": np.ascontiguousarray(vaug[slp]).reshape(ppc, P, T * 130),
            }
        )
    return in_maps


def unpack_output(res_list):
    """res_list: per-core {"out": [n_pairs, P, T, P] bf16} -> [BH, S, D] f32."""
    T = S_FULL // P
    o = np.concatenate([r["out"] for r in res_list], axis=0)  # [BH//2, P, T, P]
    o = o.reshape(BH // 2, P, T, 2, D).transpose(0, 3, 1, 2, 4)
    return np.ascontiguousarray(o).astype(np.float32).reshape(BH, S_FULL, D)


def run_sharded(q, k, v, trace=False):
    """q/k/v: [BH, S, D] fp32 numpy. Returns ([BH, S, D] fp32, results)."""
    from concourse.bass_utils import run_bass_kernel_spmd

    nc = _get_nc()
    in_maps = prep_inputs(q, k, v)
    res = run_bass_kernel_spmd(
        nc, in_maps, core_ids=list(range(N_CORES)), trace=trace
    )
    return unpack_output(res.results), res


def kernel(query, key, value, attention_mask=None):
    q = np.asarray(query, dtype=np.float32).reshape(BH, S_FULL, D)
    k = np.asarray(key, dtype=np.float32).reshape(BH, S_FULL, D)
    v = np.asarray(value, dtype=np.float32).reshape(BH, S_FULL, D)
    out, _ = run_sharded(q, k, v, trace=False)
    return out.reshape(B, H, S_FULL, D)
